# revision 11
# baseline (speedup 1.0000x reference)
"""Trainium2 Bass kernel for nn_DirectHead (retrieval_knn).

Sharding: images (Bi=256) split 32/core across 8 cores; text side replicated.
Each core computes a [Bt=256, 32] output tile; host concatenates.

Key algebra (softmax normalization cancels in both branches):
  i2t[t,i] = 0.1 * num / sqrt(q2),  p_a = mask*max(exp(20*z_a/||att_a||),1),
    z_a[t,i] = att[t,a].nvis[i] = attr_a^T (Wa^T nvis)   (v-space, no embed)
    num = sum_a p_a z_a,  q2 = sum_ab G_ab p_a p_b,  G = att att^T (DVE ttr).
  t2i[t,i] = num2 / sqrt(q2y), p2 = max(exp(20*w/||patch||),1),
    w[p,t] = x^T (Wp^T ntext)  (u-trick, v-space),
    Gp_i = x_i^T (M x^T)_i with M = Wp^T Wp (device-computed),
    ||patch||^2 = diag(Gp),  num2 = sum_p p2*w,  q2y = p2^T Gp p2.
Partition reductions (num/num2/q2/q2y) via identity/indicator matmuls into
PSUM accumulation groups. Streams images in blocks of 8 so no patch-sized
intermediate is ever materialized. All matmuls bf16 (fp32 PSUM).
"""
import sys
import numpy as np

for _p in ("/opt/trn_rl_repo",):
    if _p not in sys.path:
        sys.path.insert(0, _p)

import ml_dtypes

BF16 = ml_dtypes.bfloat16

# problem constants (hardcoded per contract)
BT = 256          # text batch
BI = 256          # image batch
NC_ = 8           # cores
IPC = BI // NC_   # images per core = 32
P = 196           # patches per image
A = 10            # attributes
V = 768           # input feature dim
D = 1024          # embed dim
KV = V // 128     # 6 v-tiles
KD = D // 128     # 8 d-tiles
NP = IPC * P      # 6272 patch tokens per core
BLK = 8           # images per streamed block
NBLK = IPC // BLK
BCOLS = BLK * P   # 1568
SCALE = 20.0

_CACHE = {}

# attribute pairs (a<=b), diagonal first so G diag cols are 0..19
_PAIRS = [(a, a) for a in range(A)] + [(a, b) for a in range(A)
                                       for b in range(a + 1, A)]


def _build():
    import concourse.bass as bass
    import concourse.tile as tile
    from concourse import bacc
    import concourse.mybir as mybir
    from concourse.masks import make_identity
    from contextlib import ExitStack

    dt = mybir.dt
    Alu = mybir.AluOpType
    Act = mybir.ActivationFunctionType

    nc = bacc.Bacc("TRN2", target_bir_lowering=False, debug=False,
                   num_devices=NC_)

    # ---- dram I/O (per-core shapes) ----
    d_x = nc.dram_tensor("patchf", [V, NP], dt.bfloat16, kind="ExternalInput").ap()
    d_attr = nc.dram_tensor("attr", [V, A * BT], dt.bfloat16, kind="ExternalInput").ap()
    d_text = nc.dram_tensor("text", [V, BT], dt.bfloat16, kind="ExternalInput").ap()
    d_vis = nc.dram_tensor("vis", [V, IPC], dt.bfloat16, kind="ExternalInput").ap()
    d_Wpdv = nc.dram_tensor("Wpdv", [D, V], dt.bfloat16, kind="ExternalInput").ap()
    d_Wadv = nc.dram_tensor("Wadv", [D, V], dt.bfloat16, kind="ExternalInput").ap()
    d_Wa = nc.dram_tensor("Wa", [V, D], dt.bfloat16, kind="ExternalInput").ap()
    d_Wt = nc.dram_tensor("Wt", [V, D], dt.bfloat16, kind="ExternalInput").ap()
    d_Wv = nc.dram_tensor("Wv", [V, D], dt.bfloat16, kind="ExternalInput").ap()
    d_mask = nc.dram_tensor("mask01", [128, 2 * A], dt.float32, kind="ExternalInput").ap()
    d_out = nc.dram_tensor("out", [BT, IPC], dt.float32, kind="ExternalOutput").ap()

    with tile.TileContext(nc) as tc, ExitStack() as top:
        const = top.enter_context(tc.tile_pool(name="const", bufs=1))
        identf = const.tile([128, 128], dt.float32)
        make_identity(nc, identf)
        identb = const.tile([128, 128], dt.bfloat16)
        nc.vector.tensor_scalar(out=identb[:], in0=identf[:], scalar1=1.0,
                                scalar2=None, op0=Alu.mult)
        # sliding indicator: IND[:, 31] = 1, else 0 -> IND[0:k, 31-i:63-i] = e_i
        IND = const.tile([128, 63], dt.bfloat16)
        nc.vector.memset(IND[:], 0.0)
        nc.vector.memset(IND[:, 31:32], 1.0)
        # diag-extraction slabs for Gp tiles [*, P]: eye at col 0 / col 128
        diagE = const.tile([128, P], dt.bfloat16)
        nc.vector.memset(diagE[:], 0.0)
        nc.vector.tensor_tensor(diagE[:, 0:128], diagE[:, 0:128], identb[:],
                                op=Alu.add)
        diagO = const.tile([128, P], dt.bfloat16)
        nc.vector.memset(diagO[:], 0.0)
        nc.vector.tensor_tensor(diagO[0:P - 128, 128:P], diagO[0:P - 128, 128:P],
                                identb[0:P - 128, 0:P - 128], op=Alu.add)
        ones1 = const.tile([1, 128], dt.float32)
        nc.vector.memset(ones1[:], 1.0)
        mask01 = const.tile([128, 2 * A], dt.float32)
        nc.sync.dma_start(mask01[:], d_mask)

        # persistent results
        res = top.enter_context(tc.tile_pool(name="res", bufs=1))
        i2t_sb = [res.tile([128, IPC], dt.float32, tag=f"i2t{t}", name=f"i2t{t}")
                  for t in range(2)]
        out_sb = [res.tile([128, IPC], dt.float32, tag=f"out{t}", name=f"out{t}")
                  for t in range(2)]
        ntextD = [res.tile([128, BT], dt.bfloat16, tag=f"nt{k}", name=f"nt{k}")
                  for k in range(KD)]

        def load_w(pool, dw, n, tagp):
            tiles = []
            for k in range(n):
                t = pool.tile([128, dw.shape[1]], dt.bfloat16, tag=f"{tagp}{k}",
                              name=f"{tagp}{k}")
                nc.sync.dma_start(t[:], dw[k * 128:(k + 1) * 128, :])
                tiles.append(t)
            return tiles

        # =========== phase 1: i2t (+ text/vis embeds) ===========
        with ExitStack() as ph:
            wpool = ph.enter_context(tc.tile_pool(name="w1", bufs=1))
            WaT = load_w(wpool, d_Wa, KV, "wa")      # [v, d] tiles
            Wadv = load_w(wpool, d_Wadv, KD, "wad")  # [d, v] tiles
            WtT = load_w(wpool, d_Wt, KV, "wt")
            WvT = load_w(wpool, d_Wv, KV, "wv")
            xa = ph.enter_context(tc.tile_pool(name="xa", bufs=1))
            attrT = load_w(xa, d_attr, KV, "attr")   # [v, a*256+t]
            textT = load_w(xa, d_text, KV, "text")   # [v, t]
            visT = load_w(xa, d_vis, KV, "vis")      # [v, i]

            psum = ph.enter_context(tc.tile_pool(name="ps1", bufs=4, space="PSUM"))
            psT = ph.enter_context(tc.tile_pool(name="psT", bufs=2, space="PSUM"))
            psB = ph.enter_context(tc.tile_pool(name="psB", bufs=1, space="PSUM"))
            tmp = ph.enter_context(tc.tile_pool(name="tmp1", bufs=4))
            gsc = ph.enter_context(tc.tile_pool(name="gsc", bufs=1))

            # ---- vis token-major -> norms -> nvisD [d, i] bf16 ----
            vtm = gsc.tile([IPC, D], dt.float32, tag="vtm", name="vtm")
            vss = tmp.tile([IPC, 2], dt.float32, tag="vss")
            vsq = gsc.tile([IPC, 512], dt.float32, tag="vsq", name="vsq")
            for nh in range(2):
                pt = psum.tile([IPC, 512], dt.float32, tag="ps")
                for k in range(KV):
                    nc.tensor.matmul(pt[:], visT[k][:], WvT[k][:, nh * 512:(nh + 1) * 512],
                                     start=(k == 0), stop=(k == KV - 1))
                nc.vector.tensor_scalar(out=vtm[:, nh * 512:(nh + 1) * 512], in0=pt[:],
                                        scalar1=1.0, scalar2=None, op0=Alu.mult)
                nc.scalar.activation(vsq[:], vtm[:, nh * 512:(nh + 1) * 512],
                                     Act.Square, accum_out=vss[:, nh:nh + 1])
            vs1 = tmp.tile([IPC, 1], dt.float32, tag="vs1")
            nc.vector.tensor_tensor(vs1[:], vss[:, 0:1], vss[:, 1:2], op=Alu.add)
            vrc = tmp.tile([IPC, 1], dt.float32, tag="vrc")
            nc.vector.reciprocal(vrc[:], vs1[:])
            vinv = tmp.tile([IPC, 1], dt.float32, tag="vinv")
            nc.scalar.activation(vinv[:], vrc[:], Act.Sqrt)  # 1/||vis||
            nvis_tm = gsc.tile([IPC, D], dt.bfloat16, tag="nvtm", name="nvtm")
            nc.vector.tensor_scalar(out=nvis_tm[:], in0=vtm[:], scalar1=vinv[:, 0:1],
                                    scalar2=None, op0=Alu.mult)
            nvisD = []
            for k in range(KD):
                ptt = psT.tile([128, IPC], dt.bfloat16, tag="psT")
                nc.tensor.transpose(ptt[:], nvis_tm[:, k * 128:(k + 1) * 128],
                                    identb[0:IPC, 0:IPC])
                t = gsc.tile([128, IPC], dt.bfloat16, tag=f"nv{k}", name=f"nv{k}")
                nc.scalar.copy(t[:], ptt[:])
                nvisD.append(t)

            # ---- text: token-major norms + feature-major ntextD ----
            tss = tmp.tile([128, 4], dt.float32, tag="tss")
            for th in range(2):
                for nh in range(2):
                    pt = psum.tile([128, 512], dt.float32, tag="ps")
                    for k in range(KV):
                        nc.tensor.matmul(pt[:], textT[k][:, th * 128:(th + 1) * 128],
                                         WtT[k][:, nh * 512:(nh + 1) * 512],
                                         start=(k == 0), stop=(k == KV - 1))
                    sq = gsc.tile([128, 512], dt.float32, tag="tsq", name="tsq2",
                                  bufs=2)
                    nc.scalar.activation(sq[:], pt[:], Act.Square,
                                         accum_out=tss[:, 2 * th + nh:2 * th + nh + 1])
            invnt = tmp.tile([128, 2], dt.float32, tag="invnt")
            for th in range(2):
                s1 = tmp.tile([128, 1], dt.float32, tag="s1")
                nc.vector.tensor_tensor(s1[:], tss[:, 2 * th:2 * th + 1],
                                        tss[:, 2 * th + 1:2 * th + 2], op=Alu.add)
                s2 = tmp.tile([128, 1], dt.float32, tag="s2")
                nc.vector.reciprocal(s2[:], s1[:])
                nc.scalar.activation(invnt[:, th:th + 1], s2[:], Act.Sqrt)
            # broadcast inv_t over d-partitions: bct [128, 256]
            bct = gsc.tile([128, BT], dt.float32, tag="bct", name="bct")
            for th in range(2):
                tp = psT.tile([1, 128], dt.float32, tag="psT")
                nc.tensor.transpose(tp[:], invnt[:, th:th + 1], identf[:])
                tps = tmp.tile([1, 128], dt.float32, tag="tps")
                nc.scalar.copy(tps[:], tp[:])
                bp = psum.tile([128, 128], dt.float32, tag="ps")
                nc.tensor.matmul(bp[:], ones1[:], tps[:], start=True, stop=True)
                nc.vector.tensor_scalar(out=bct[:, th * 128:(th + 1) * 128], in0=bp[:],
                                        scalar1=1.0, scalar2=None, op0=Alu.mult)
            # feature-major text embed scaled: ntextD[d, t]
            for m in range(KD):
                pt = psum.tile([128, BT], dt.float32, tag="ps")
                for k in range(KV):
                    nc.tensor.matmul(pt[:], WtT[k][:, m * 128:(m + 1) * 128],
                                     textT[k][:], start=(k == 0), stop=(k == KV - 1))
                nc.vector.tensor_tensor(ntextD[m][:], pt[:], bct[:], op=Alu.mult)

            # ---- att token-major embed (for Gram G via DVE ttr) ----
            atm_pool = ph.enter_context(tc.tile_pool(name="atm", bufs=1))
            att_tm = []
            for j in range(2 * A):
                # k outer / nh inner: each attrT lhsT slice loaded once for 2 mm
                sb = atm_pool.tile([128, D], dt.bfloat16, tag=f"atm{j}", name=f"atm{j}")
                pts = [psum.tile([128, 512], dt.float32, tag="ps", name=f"pa{nh}")
                       for nh in range(2)]
                for k in range(KV):
                    for nh in range(2):
                        nc.tensor.matmul(pts[nh][:], attrT[k][:, j * 128:(j + 1) * 128],
                                         WaT[k][:, nh * 512:(nh + 1) * 512],
                                         start=(k == 0), stop=(k == KV - 1))
                nc.vector.tensor_scalar(out=sb[:, 0:512], in0=pts[0][:],
                                        scalar1=1.0, scalar2=None, op0=Alu.mult)
                nc.scalar.copy(sb[:, 512:1024], pts[1][:])
                att_tm.append(sb)
            # G2[t, 2*pair+th]: diag pairs are cols 0..19
            G2 = gsc.tile([128, 2 * len(_PAIRS)], dt.float32, tag="G2", name="G2")
            for pi, (a, b) in enumerate(_PAIRS):
                for th in range(2):
                    idx = 2 * pi + th
                    scr = gsc.tile([128, D], dt.bfloat16, tag="gscr", name="gscr",
                                   bufs=4)
                    meng = nc.gpsimd if idx % 4 == 3 else nc.vector
                    meng.tensor_tensor(scr[:], att_tm[2 * a + th][:],
                                       att_tm[2 * b + th][:], op=Alu.mult)
                    if idx % 2 == 0:
                        nc.vector.tensor_reduce(G2[:, idx:idx + 1], scr[:],
                                                axis=mybir.AxisListType.X, op=Alu.add)
                    else:
                        scr2 = gsc.tile([128, D], dt.bfloat16, tag="gscr2",
                                        name="gscr2", bufs=4)
                        nc.scalar.activation(scr2[:], scr[:], Act.Copy,
                                             accum_out=G2[:, idx:idx + 1])
            # inv_na20[t, j=(a,th)] = 20/sqrt(G_aa) = sqrt(400/G_aa)
            invr = tmp.tile([128, 2 * A], dt.float32, tag="invr")
            nc.vector.reciprocal(invr[:], G2[:, 0:2 * A])
            invna = gsc.tile([128, 2 * A], dt.float32, tag="invna", name="invna")
            nc.scalar.activation(invna[:], invr[:], Act.Sqrt, scale=SCALE * SCALE)
            # double offdiag G columns (q2 = sum over unordered pairs)
            G2d = gsc.tile([128, 2 * len(_PAIRS)], dt.float32, tag="G2d", name="G2d")
            nc.vector.tensor_scalar(out=G2d[:, 0:2 * A], in0=G2[:, 0:2 * A],
                                    scalar1=1.0, scalar2=None, op0=Alu.mult)
            nc.vector.tensor_scalar(out=G2d[:, 2 * A:], in0=G2[:, 2 * A:],
                                    scalar1=2.0, scalar2=None, op0=Alu.mult)

            # ---- z via v-space: uz = Wa_dv^T nvisD ; z_j = attr_j^T uz ----
            uz = []
            for k in range(KV):
                pt = psum.tile([128, IPC], dt.float32, tag="ps")
                for kd in range(KD):
                    nc.tensor.matmul(pt[:], Wadv[kd][:, k * 128:(k + 1) * 128],
                                     nvisD[kd][:], start=(kd == 0), stop=(kd == KD - 1))
                t = gsc.tile([128, IPC], dt.bfloat16, tag=f"uz{k}", name=f"uz{k}")
                nc.scalar.copy(t[:], pt[:])
                uz.append(t)
            ppool = ph.enter_context(tc.tile_pool(name="pp", bufs=1))
            p_sb, g_sb = [], []
            for j in range(2 * A):
                zp = psum.tile([128, IPC], dt.float32, tag="ps")
                for k in range(KV):
                    nc.tensor.matmul(zp[:], attrT[k][:, j * 128:(j + 1) * 128],
                                     uz[k][:], start=(k == 0), stop=(k == KV - 1))
                e = tmp.tile([128, IPC], dt.float32, tag="eA")
                nc.scalar.activation(e[:], zp[:], Act.Exp, scale=invna[:, j:j + 1])
                p = ppool.tile([128, IPC], dt.float32, tag=f"p{j}", name=f"p{j}")
                nc.vector.tensor_scalar(out=p[:], in0=e[:], scalar1=1.0,
                                        scalar2=mask01[:, j:j + 1], op0=Alu.max,
                                        op1=Alu.mult)
                g = ppool.tile([128, IPC], dt.bfloat16, tag=f"g{j}", name=f"g{j}")
                nc.vector.tensor_tensor(g[:], p[:], zp[:], op=Alu.mult)
                p_sb.append(p)
                g_sb.append(g)
            # num[th] = sum_a g ; q2[th] = sum_pairs Gc*p_a*p_b (identity-mm)
            for th in range(2):
                nump = psB.tile([128, IPC], dt.float32, tag="numP")
                for a in range(A):
                    nc.tensor.matmul(nump[:], identb[:], g_sb[2 * a + th][:],
                                     start=(a == 0), stop=(a == A - 1))
                q2p = psB.tile([128, IPC], dt.float32, tag="q2P")
                with tc.tile_pool(name=f"rp{th}", bufs=4) as rp:
                    for pi, (a, b) in enumerate(_PAIRS):
                        r = rp.tile([128, IPC], dt.bfloat16, tag="rab")
                        nc.vector.scalar_tensor_tensor(
                            out=r[:], in0=p_sb[2 * a + th][:],
                            scalar=G2d[:, 2 * pi + th:2 * pi + th + 1],
                            in1=p_sb[2 * b + th][:], op0=Alu.mult, op1=Alu.mult)
                        nc.tensor.matmul(q2p[:], identb[:], r[:], start=(pi == 0),
                                         stop=(pi == len(_PAIRS) - 1))
                rcq = tmp.tile([128, IPC], dt.float32, tag="rcq")
                nc.vector.reciprocal(rcq[:], q2p[:])
                sq = tmp.tile([128, IPC], dt.float32, tag="sqq")
                nc.scalar.activation(sq[:], rcq[:], Act.Sqrt)  # 1/sqrt(q2)
                nm = tmp.tile([128, IPC], dt.float32, tag="nm")
                nc.vector.tensor_scalar(out=nm[:], in0=nump[:], scalar1=0.1,
                                        scalar2=None, op0=Alu.mult)
                nc.vector.tensor_tensor(i2t_sb[th][:], nm[:], sq[:], op=Alu.mult)

        # =========== phase 2: t2i (streamed blocks of BLK images) ===========
        xpool = top.enter_context(tc.tile_pool(name="xp", bufs=1))
        x_fm = [xpool.tile([128, NP], dt.bfloat16, tag=f"x{k}", name=f"x{k}")
                for k in range(KV)]
        for k in range(KV):
            nc.sync.dma_start(x_fm[k][:], d_x[k * 128:(k + 1) * 128, :])

        with ExitStack() as ph:
            upool = ph.enter_context(tc.tile_pool(name="up", bufs=1))
            psum = ph.enter_context(tc.tile_pool(name="ps2", bufs=6, space="PSUM"))
            psN = ph.enter_context(tc.tile_pool(name="psN", bufs=1, space="PSUM"))
            tmp = ph.enter_context(tc.tile_pool(name="tmp2", bufs=4))
            sp = ph.enter_context(tc.tile_pool(name="sp", bufs=1))

            with tc.tile_pool(name="wpd", bufs=1) as wpd:
                Wpdv = load_w(wpd, d_Wpdv, KD, "wpd")  # [d, v] tiles
                # M = Wp_dv^T Wp_dv  [v, v'] bf16
                M_sb = [upool.tile([128, V], dt.bfloat16, tag=f"M{m}", name=f"M{m}")
                        for m in range(KV)]
                for m in range(KV):
                    for c in range(2):
                        pt = psum.tile([128, 512], dt.float32, tag="ps")
                        for k in range(KD):
                            nc.tensor.matmul(pt[:, 0:384],
                                             Wpdv[k][:, m * 128:(m + 1) * 128],
                                             Wpdv[k][:, c * 384:(c + 1) * 384],
                                             start=(k == 0), stop=(k == KD - 1))
                        nc.vector.tensor_scalar(out=M_sb[m][:, c * 384:(c + 1) * 384],
                                                in0=pt[:, 0:384], scalar1=1.0,
                                                scalar2=None, op0=Alu.mult)
                # u = Wp_dv^T ntextD  [v, t] bf16
                u_sb = [upool.tile([128, BT], dt.bfloat16, tag=f"u{m}", name=f"u{m}")
                        for m in range(KV)]
                for m in range(KV):
                    pt = psum.tile([128, BT], dt.float32, tag="ps")
                    for k in range(KD):
                        nc.tensor.matmul(pt[:], Wpdv[k][:, m * 128:(m + 1) * 128],
                                         ntextD[k][:], start=(k == 0), stop=(k == KD - 1))
                    nc.scalar.copy(u_sb[m][:], pt[:])

            num2p = psN.tile([IPC, BT], dt.float32, tag="num2")
            q2yp = psN.tile([IPC, BT], dt.float32, tag="q2y")

            for b in range(NBLK):
                b0 = b * BCOLS
                # t2 block = M x[:, block]  [v', 1568] bf16
                t2 = [sp.tile([128, BCOLS], dt.bfloat16, tag=f"t2{m}", name=f"t2{m}",
                              bufs=2) for m in range(KV)]
                ci = 0
                n0 = 0
                while n0 < BCOLS:
                    nw = min(512, BCOLS - n0)
                    for m in range(KV):
                        pt = psum.tile([128, 512], dt.float32, tag="ps")
                        for k in range(KV):
                            nc.tensor.matmul(pt[:, 0:nw],
                                             M_sb[k][:, m * 128:(m + 1) * 128],
                                             x_fm[k][:, b0 + n0:b0 + n0 + nw],
                                             start=(k == 0), stop=(k == KV - 1))
                        if ci % 2 == 0:
                            nc.vector.tensor_scalar(out=t2[m][:, n0:n0 + nw],
                                                    in0=pt[:, 0:nw], scalar1=1.0,
                                                    scalar2=None, op0=Alu.mult)
                        else:
                            nc.scalar.copy(t2[m][:, n0:n0 + nw], pt[:, 0:nw])
                        ci += 1
                    n0 += nw
                # Gp per image + diag
                Gp = [sp.tile([128, P], dt.bfloat16, tag=f"gp{t}", name=f"gp{t}",
                              bufs=2) for t in range(2 * BLK)]
                dcol = sp.tile([128, 2 * BLK], dt.float32, tag="dcol", name="dcol",
                               bufs=2)
                nc.gpsimd.memset(dcol[:], 1.0)
                for ib in range(BLK):
                    c0 = b0 + ib * P
                    cb = ib * P
                    for mt in range(2):
                        moff = mt * 128
                        mw = 128 if mt == 0 else P - 128
                        tau = 2 * ib + mt
                        pt = psum.tile([128, 512], dt.float32, tag="ps")
                        for k in range(KV):
                            nc.tensor.matmul(pt[0:mw, 0:P],
                                             x_fm[k][:, c0 + moff:c0 + moff + mw],
                                             t2[k][:, cb:cb + P],
                                             start=(k == 0), stop=(k == KV - 1))
                        if tau % 2 == 0:
                            nc.vector.tensor_scalar(out=Gp[tau][0:mw, :],
                                                    in0=pt[0:mw, 0:P], scalar1=1.0,
                                                    scalar2=None, op0=Alu.mult)
                        else:
                            nc.scalar.copy(Gp[tau][0:mw, :], pt[0:mw, 0:P])
                        dsc = tmp.tile([128, P], dt.bfloat16, tag="dsc", bufs=3)
                        dI = diagE if mt == 0 else diagO
                        meng = nc.vector if tau % 2 == 0 else nc.gpsimd
                        meng.tensor_tensor(dsc[0:mw, :], Gp[tau][0:mw, :],
                                           dI[0:mw, :], op=Alu.mult)
                        if tau % 2 == 0:
                            nc.vector.tensor_reduce(dcol[0:mw, tau:tau + 1],
                                                    dsc[0:mw, :],
                                                    axis=mybir.AxisListType.X,
                                                    op=Alu.add)
                        else:
                            dsc2 = tmp.tile([128, P], dt.bfloat16, tag="dsc2", bufs=3)
                            nc.scalar.activation(dsc2[0:mw, :], dsc[0:mw, :], Act.Copy,
                                                 accum_out=dcol[0:mw, tau:tau + 1])
                # inv20 = 20/sqrt(max(diag,eps)) = sqrt(400*recip(max(diag,eps)))
                dm = tmp.tile([128, 2 * BLK], dt.float32, tag="dm", bufs=2)
                nc.vector.tensor_scalar(out=dm[:], in0=dcol[:], scalar1=1e-12,
                                        scalar2=None, op0=Alu.max)
                dr = tmp.tile([128, 2 * BLK], dt.float32, tag="dr", bufs=2)
                nc.vector.reciprocal(dr[:], dm[:])
                inv20 = tmp.tile([128, 2 * BLK], dt.float32, tag="i20", bufs=2)
                nc.scalar.activation(inv20[:], dr[:], Act.Sqrt, scale=SCALE * SCALE)

                # w -> e/p2/g -> num2 ; r -> g2 -> q2y
                for ib in range(BLK):
                    c0 = b0 + ib * P
                    i = b * BLK + ib
                    es, p2s = [], []
                    for mt in range(2):
                        moff = mt * 128
                        mw = 128 if mt == 0 else P - 128
                        tau = 2 * ib + mt
                        wp = psum.tile([128, 512], dt.float32, tag="ps")
                        for k in range(KV):
                            nc.tensor.matmul(wp[0:mw, 0:BT],
                                             x_fm[k][:, c0 + moff:c0 + moff + mw],
                                             u_sb[k][:], start=(k == 0),
                                             stop=(k == KV - 1))
                        e = tmp.tile([128, BT], dt.float32, tag="e2", bufs=6)
                        nc.scalar.activation(e[0:mw, :], wp[0:mw, 0:BT], Act.Exp,
                                             scale=inv20[0:mw, tau:tau + 1])
                        p2 = sp.tile([128, BT], dt.bfloat16, tag="p2", bufs=6)
                        nc.gpsimd.tensor_scalar(out=p2[0:mw, :], in0=e[0:mw, :],
                                                scalar1=1.0, scalar2=None, op0=Alu.max)
                        g = sp.tile([128, BT], dt.bfloat16, tag="gg", bufs=4)
                        nc.vector.scalar_tensor_tensor(out=g[0:mw, :], in0=e[0:mw, :],
                                                       scalar=1.0, in1=wp[0:mw, 0:BT],
                                                       op0=Alu.max, op1=Alu.mult)
                        nc.tensor.matmul(num2p[:], IND[0:mw, 31 - i:63 - i], g[0:mw, :],
                                         start=(i == 0 and mt == 0),
                                         stop=(i == IPC - 1 and mt == 1))
                        es.append(e)
                        p2s.append(p2)
                    for mt in range(2):
                        moff = mt * 128
                        mw = 128 if mt == 0 else P - 128
                        rp = psum.tile([128, 512], dt.float32, tag="ps")
                        for k in range(2):
                            kw = 128 if k == 0 else P - 128
                            nc.tensor.matmul(rp[0:mw, 0:BT],
                                             Gp[2 * ib + k][0:kw, moff:moff + mw],
                                             p2s[k][0:kw, :], start=(k == 0),
                                             stop=(k == 1))
                        g2 = sp.tile([128, BT], dt.bfloat16, tag="g2", bufs=4)
                        nc.vector.scalar_tensor_tensor(out=g2[0:mw, :],
                                                       in0=es[mt][0:mw, :], scalar=1.0,
                                                       in1=rp[0:mw, 0:BT],
                                                       op0=Alu.max, op1=Alu.mult)
                        nc.tensor.matmul(q2yp[:], IND[0:mw, 31 - i:63 - i], g2[0:mw, :],
                                         start=(i == 0 and mt == 0),
                                         stop=(i == IPC - 1 and mt == 1))

            # t2i = num2/sqrt(q2y)  [IPC, BT] -> transpose -> out
            rc = tmp.tile([IPC, BT], dt.float32, tag="rcy")
            nc.vector.reciprocal(rc[:], q2yp[:])
            sq = tmp.tile([IPC, BT], dt.float32, tag="sqy")
            nc.scalar.activation(sq[:], rc[:], Act.Sqrt)  # 1/sqrt(q2y)
            t2i = tmp.tile([IPC, BT], dt.float32, tag="t2i")
            nc.vector.tensor_tensor(t2i[:], num2p[:], sq[:], op=Alu.mult)
            for th in range(2):
                tp = psum.tile([128, 512], dt.float32, tag="ps")
                nc.tensor.transpose(tp[0:128, 0:IPC], t2i[:, th * 128:(th + 1) * 128],
                                    identf[0:IPC, 0:IPC])
                nc.vector.tensor_tensor(out_sb[th][:], i2t_sb[th][:], tp[0:128, 0:IPC],
                                        op=Alu.add)
                nc.sync.dma_start(d_out[th * 128:(th + 1) * 128, :], out_sb[th][:])

    nc.compile()
    return nc


def _prep(inputs):
    vf = np.asarray(inputs["visual_feature"], np.float32)
    tf = np.asarray(inputs["textual_feature"], np.float32)
    af = np.asarray(inputs["attribute_feature"], np.float32)
    an = np.asarray(inputs["att_nums"]).astype(np.int64)
    Wp = np.asarray(inputs["Wp"], np.float32)
    Wa = np.asarray(inputs["Wa"], np.float32)
    Wt = np.asarray(inputs["Wt"], np.float32)
    Wv = np.asarray(inputs["Wv"], np.float32)

    textT = np.ascontiguousarray(tf.T).astype(BF16)                       # [768,256]
    attrT = np.ascontiguousarray(af.transpose(1, 0, 2).reshape(A * BT, V).T).astype(BF16)
    WaT = np.ascontiguousarray(Wa.T).astype(BF16)
    WtT = np.ascontiguousarray(Wt.T).astype(BF16)
    WvT = np.ascontiguousarray(Wv.T).astype(BF16)
    Wpdv = np.ascontiguousarray(Wp).astype(BF16)                          # [1024,768]
    Wadv = np.ascontiguousarray(Wa).astype(BF16)
    # mask01 [128, 20]: col j=(a, th) -> 1.0 if a < att_nums[th*128+r]
    m = (np.arange(A)[None, :] < an[:, None]).astype(np.float32)          # [256,10]
    mask01 = np.empty((128, 2 * A), np.float32)
    for a in range(A):
        for th in range(2):
            mask01[:, 2 * a + th] = m[th * 128:(th + 1) * 128, a]

    maps = []
    for c in range(NC_):
        sl = slice(c * IPC, (c + 1) * IPC)
        pat = vf[sl, 1:, :]                                               # [32,196,768]
        patf = np.ascontiguousarray(pat.reshape(NP, V).T).astype(BF16)
        visT = np.ascontiguousarray(vf[sl, 0, :].T).astype(BF16)
        maps.append({
            "patchf": patf, "attr": attrT, "text": textT, "vis": visT,
            "Wpdv": Wpdv, "Wadv": Wadv, "Wa": WaT, "Wt": WtT, "Wv": WvT,
            "mask01": mask01,
        })
    return maps


def _run(inputs, trace=False):
    from concourse.bass_utils import run_bass_kernel_spmd
    if "nc" not in _CACHE:
        _CACHE["nc"] = _build()
    maps = _prep(inputs)
    res = run_bass_kernel_spmd(_CACHE["nc"], maps, list(range(NC_)), trace=trace)
    out = np.concatenate([res.results[c]["out"] for c in range(NC_)], axis=1)
    return out.astype(np.float32), res


def kernel(**inputs):
    out, _ = _run(inputs, trace=False)
    return out


# revision 18
# speedup vs baseline: 1.0053x; 1.0053x over previous
"""Trainium2 Bass kernel for nn_DirectHead (retrieval_knn).

Sharding: images (Bi=256) split 32/core across 8 cores; text side replicated.
Each core computes a [Bt=256, 32] output tile; host concatenates.

Key algebra (softmax normalization cancels in both branches):
  i2t[t,i] = 0.1 * num / sqrt(q2),  p_a = mask*max(exp(20*z_a/||att_a||),1),
    z_a[t,i] = att[t,a].nvis[i] = attr_a^T (Wa^T nvis)   (v-space, no embed)
    num = sum_a p_a z_a,  q2 = sum_ab G_ab p_a p_b,  G = att att^T (DVE ttr).
  t2i[t,i] = num2 / sqrt(q2y), p2 = max(exp(20*w/||patch||),1),
    w[p,t] = x^T (Wp^T ntext)  (u-trick, v-space),
    Gp_i = x_i^T (M x^T)_i with M = Wp^T Wp (device-computed),
    ||patch||^2 = diag(Gp),  num2 = sum_p p2*w,  q2y = p2^T Gp p2.
Partition reductions (num/num2/q2/q2y) via identity/indicator matmuls into
PSUM accumulation groups. Streams images in blocks of 8 so no patch-sized
intermediate is ever materialized. All matmuls bf16 (fp32 PSUM).
"""
import sys
import numpy as np

for _p in ("/opt/trn_rl_repo",):
    if _p not in sys.path:
        sys.path.insert(0, _p)

import ml_dtypes

BF16 = ml_dtypes.bfloat16

# problem constants (hardcoded per contract)
BT = 256          # text batch
BI = 256          # image batch
NC_ = 8           # cores
IPC = BI // NC_   # images per core = 32
P = 196           # patches per image
A = 10            # attributes
V = 768           # input feature dim
D = 1024          # embed dim
KV = V // 128     # 6 v-tiles
KD = D // 128     # 8 d-tiles
NP = IPC * P      # 6272 patch tokens per core
BLK = 16          # images per streamed block
NBLK = IPC // BLK
BCOLS = BLK * P   # 1568
SCALE = 20.0

_CACHE = {}

# attribute pairs (a<=b), diagonal first so G diag cols are 0..19
_PAIRS = [(a, a) for a in range(A)] + [(a, b) for a in range(A)
                                       for b in range(a + 1, A)]


def _build():
    import concourse.bass as bass
    import concourse.tile as tile
    from concourse import bacc
    import concourse.mybir as mybir
    from concourse.masks import make_identity
    from contextlib import ExitStack

    dt = mybir.dt
    Alu = mybir.AluOpType
    Act = mybir.ActivationFunctionType

    nc = bacc.Bacc("TRN2", target_bir_lowering=False, debug=False,
                   num_devices=NC_)

    # ---- dram I/O (per-core shapes) ----
    d_x = nc.dram_tensor("patchf", [V, NP], dt.bfloat16, kind="ExternalInput").ap()
    d_attr = nc.dram_tensor("attr", [V, A * BT], dt.bfloat16, kind="ExternalInput").ap()
    d_text = nc.dram_tensor("text", [V, BT], dt.bfloat16, kind="ExternalInput").ap()
    d_vis = nc.dram_tensor("vis", [V, IPC], dt.bfloat16, kind="ExternalInput").ap()
    d_Wpdv = nc.dram_tensor("Wpdv", [D, V], dt.bfloat16, kind="ExternalInput").ap()
    d_Wadv = nc.dram_tensor("Wadv", [D, V], dt.bfloat16, kind="ExternalInput").ap()
    d_Wa = nc.dram_tensor("Wa", [V, D], dt.bfloat16, kind="ExternalInput").ap()
    d_Wt = nc.dram_tensor("Wt", [V, D], dt.bfloat16, kind="ExternalInput").ap()
    d_Wv = nc.dram_tensor("Wv", [V, D], dt.bfloat16, kind="ExternalInput").ap()
    d_mask = nc.dram_tensor("mask01", [128, 2 * A], dt.float32, kind="ExternalInput").ap()
    d_out = nc.dram_tensor("out", [BT, IPC], dt.float32, kind="ExternalOutput").ap()

    with tile.TileContext(nc) as tc, ExitStack() as top:
        const = top.enter_context(tc.tile_pool(name="const", bufs=1))
        identf = const.tile([128, 128], dt.float32)
        make_identity(nc, identf)
        identb = const.tile([128, 128], dt.bfloat16)
        nc.vector.tensor_scalar(out=identb[:], in0=identf[:], scalar1=1.0,
                                scalar2=None, op0=Alu.mult)
        # sliding indicator: IND[:, 31] = 1, else 0 -> IND[0:k, 31-i:63-i] = e_i
        IND = const.tile([128, 63], dt.bfloat16)
        nc.vector.memset(IND[:], 0.0)
        nc.vector.memset(IND[:, 31:32], 1.0)
        # diag-extraction slabs for Gp tiles [*, P]: eye at col 0 / col 128
        diagE = const.tile([128, P], dt.bfloat16)
        nc.vector.memset(diagE[:], 0.0)
        nc.vector.tensor_tensor(diagE[:, 0:128], diagE[:, 0:128], identb[:],
                                op=Alu.add)
        diagO = const.tile([128, P], dt.bfloat16)
        nc.vector.memset(diagO[:], 0.0)
        nc.vector.tensor_tensor(diagO[0:P - 128, 128:P], diagO[0:P - 128, 128:P],
                                identb[0:P - 128, 0:P - 128], op=Alu.add)
        ones1 = const.tile([1, 128], dt.float32)
        nc.vector.memset(ones1[:], 1.0)
        mask01 = const.tile([128, 2 * A], dt.float32)
        nc.sync.dma_start(mask01[:], d_mask)

        # persistent results
        res = top.enter_context(tc.tile_pool(name="res", bufs=1))
        i2t_sb = [res.tile([128, IPC], dt.float32, tag=f"i2t{t}", name=f"i2t{t}")
                  for t in range(2)]
        out_sb = [res.tile([128, IPC], dt.float32, tag=f"out{t}", name=f"out{t}")
                  for t in range(2)]
        ntextD = [res.tile([128, BT], dt.bfloat16, tag=f"nt{k}", name=f"nt{k}")
                  for k in range(KD)]

        def load_w(pool, dw, n, tagp):
            tiles = []
            for k in range(n):
                t = pool.tile([128, dw.shape[1]], dt.bfloat16, tag=f"{tagp}{k}",
                              name=f"{tagp}{k}")
                nc.sync.dma_start(t[:], dw[k * 128:(k + 1) * 128, :])
                tiles.append(t)
            return tiles

        # =========== phase 1: i2t (+ text/vis embeds) ===========
        with ExitStack() as ph:
            wpool = ph.enter_context(tc.tile_pool(name="w1", bufs=1))
            WaT = load_w(wpool, d_Wa, KV, "wa")      # [v, d] tiles
            Wadv = load_w(wpool, d_Wadv, KD, "wad")  # [d, v] tiles
            WtT = load_w(wpool, d_Wt, KV, "wt")
            WvT = load_w(wpool, d_Wv, KV, "wv")
            xa = ph.enter_context(tc.tile_pool(name="xa", bufs=1))
            attrT = load_w(xa, d_attr, KV, "attr")   # [v, a*256+t]
            textT = load_w(xa, d_text, KV, "text")   # [v, t]
            visT = load_w(xa, d_vis, KV, "vis")      # [v, i]

            psum = ph.enter_context(tc.tile_pool(name="ps1", bufs=4, space="PSUM"))
            psT = ph.enter_context(tc.tile_pool(name="psT", bufs=2, space="PSUM"))
            psB = ph.enter_context(tc.tile_pool(name="psB", bufs=1, space="PSUM"))
            tmp = ph.enter_context(tc.tile_pool(name="tmp1", bufs=4))
            gsc = ph.enter_context(tc.tile_pool(name="gsc", bufs=1))

            # ---- vis token-major -> norms -> nvisD [d, i] bf16 ----
            vtm = gsc.tile([IPC, D], dt.float32, tag="vtm", name="vtm")
            vss = tmp.tile([IPC, 2], dt.float32, tag="vss")
            vsq = gsc.tile([IPC, 512], dt.float32, tag="vsq", name="vsq")
            for nh in range(2):
                pt = psum.tile([IPC, 512], dt.float32, tag="ps")
                for k in range(KV):
                    nc.tensor.matmul(pt[:], visT[k][:], WvT[k][:, nh * 512:(nh + 1) * 512],
                                     start=(k == 0), stop=(k == KV - 1))
                nc.vector.tensor_scalar(out=vtm[:, nh * 512:(nh + 1) * 512], in0=pt[:],
                                        scalar1=1.0, scalar2=None, op0=Alu.mult)
                nc.scalar.activation(vsq[:], vtm[:, nh * 512:(nh + 1) * 512],
                                     Act.Square, accum_out=vss[:, nh:nh + 1])
            vs1 = tmp.tile([IPC, 1], dt.float32, tag="vs1")
            nc.vector.tensor_tensor(vs1[:], vss[:, 0:1], vss[:, 1:2], op=Alu.add)
            vrc = tmp.tile([IPC, 1], dt.float32, tag="vrc")
            nc.vector.reciprocal(vrc[:], vs1[:])
            vinv = tmp.tile([IPC, 1], dt.float32, tag="vinv")
            nc.scalar.activation(vinv[:], vrc[:], Act.Sqrt)  # 1/||vis||
            nvis_tm = gsc.tile([IPC, D], dt.bfloat16, tag="nvtm", name="nvtm")
            nc.vector.tensor_scalar(out=nvis_tm[:], in0=vtm[:], scalar1=vinv[:, 0:1],
                                    scalar2=None, op0=Alu.mult)
            nvisD = []
            for k in range(KD):
                ptt = psT.tile([128, IPC], dt.bfloat16, tag="psT")
                nc.tensor.transpose(ptt[:], nvis_tm[:, k * 128:(k + 1) * 128],
                                    identb[0:IPC, 0:IPC])
                t = gsc.tile([128, IPC], dt.bfloat16, tag=f"nv{k}", name=f"nv{k}")
                nc.scalar.copy(t[:], ptt[:])
                nvisD.append(t)

            # ---- text: token-major norms + feature-major ntextD ----
            tss = tmp.tile([128, 4], dt.float32, tag="tss")
            for th in range(2):
                for nh in range(2):
                    pt = psum.tile([128, 512], dt.float32, tag="ps")
                    for k in range(KV):
                        nc.tensor.matmul(pt[:], textT[k][:, th * 128:(th + 1) * 128],
                                         WtT[k][:, nh * 512:(nh + 1) * 512],
                                         start=(k == 0), stop=(k == KV - 1))
                    sq = gsc.tile([128, 512], dt.float32, tag="tsq", name="tsq2",
                                  bufs=2)
                    nc.scalar.activation(sq[:], pt[:], Act.Square,
                                         accum_out=tss[:, 2 * th + nh:2 * th + nh + 1])
            invnt = tmp.tile([128, 2], dt.float32, tag="invnt")
            for th in range(2):
                s1 = tmp.tile([128, 1], dt.float32, tag="s1")
                nc.vector.tensor_tensor(s1[:], tss[:, 2 * th:2 * th + 1],
                                        tss[:, 2 * th + 1:2 * th + 2], op=Alu.add)
                s2 = tmp.tile([128, 1], dt.float32, tag="s2")
                nc.vector.reciprocal(s2[:], s1[:])
                nc.scalar.activation(invnt[:, th:th + 1], s2[:], Act.Sqrt)
            # broadcast inv_t over d-partitions: bct [128, 256]
            bct = gsc.tile([128, BT], dt.float32, tag="bct", name="bct")
            for th in range(2):
                tp = psT.tile([1, 128], dt.float32, tag="psT")
                nc.tensor.transpose(tp[:], invnt[:, th:th + 1], identf[:])
                tps = tmp.tile([1, 128], dt.float32, tag="tps")
                nc.scalar.copy(tps[:], tp[:])
                bp = psum.tile([128, 128], dt.float32, tag="ps")
                nc.tensor.matmul(bp[:], ones1[:], tps[:], start=True, stop=True)
                nc.vector.tensor_scalar(out=bct[:, th * 128:(th + 1) * 128], in0=bp[:],
                                        scalar1=1.0, scalar2=None, op0=Alu.mult)
            # feature-major text embed scaled: ntextD[d, t]
            for m in range(KD):
                pt = psum.tile([128, BT], dt.float32, tag="ps")
                for k in range(KV):
                    nc.tensor.matmul(pt[:], WtT[k][:, m * 128:(m + 1) * 128],
                                     textT[k][:], start=(k == 0), stop=(k == KV - 1))
                nc.vector.tensor_tensor(ntextD[m][:], pt[:], bct[:], op=Alu.mult)

            # ---- att token-major embed (for Gram G via DVE ttr) ----
            atm_pool = ph.enter_context(tc.tile_pool(name="atm", bufs=1))
            att_tm = []
            for j in range(2 * A):
                # k outer / nh inner: each attrT lhsT slice loaded once for 2 mm
                sb = atm_pool.tile([128, D], dt.bfloat16, tag=f"atm{j}", name=f"atm{j}")
                pts = [psum.tile([128, 512], dt.float32, tag="ps", name=f"pa{nh}")
                       for nh in range(2)]
                for k in range(KV):
                    for nh in range(2):
                        nc.tensor.matmul(pts[nh][:], attrT[k][:, j * 128:(j + 1) * 128],
                                         WaT[k][:, nh * 512:(nh + 1) * 512],
                                         start=(k == 0), stop=(k == KV - 1))
                nc.vector.tensor_scalar(out=sb[:, 0:512], in0=pts[0][:],
                                        scalar1=1.0, scalar2=None, op0=Alu.mult)
                nc.scalar.copy(sb[:, 512:1024], pts[1][:])
                att_tm.append(sb)
            # G2[t, 2*pair+th]: diag pairs are cols 0..19
            G2 = gsc.tile([128, 2 * len(_PAIRS)], dt.float32, tag="G2", name="G2")
            for pi, (a, b) in enumerate(_PAIRS):
                for th in range(2):
                    idx = 2 * pi + th
                    scr = gsc.tile([128, D], dt.bfloat16, tag="gscr", name="gscr",
                                   bufs=4)
                    meng = nc.gpsimd if idx % 4 == 3 else nc.vector
                    meng.tensor_tensor(scr[:], att_tm[2 * a + th][:],
                                       att_tm[2 * b + th][:], op=Alu.mult)
                    if idx % 2 == 0:
                        nc.vector.tensor_reduce(G2[:, idx:idx + 1], scr[:],
                                                axis=mybir.AxisListType.X, op=Alu.add)
                    else:
                        scr2 = gsc.tile([128, D], dt.bfloat16, tag="gscr2",
                                        name="gscr2", bufs=4)
                        nc.scalar.activation(scr2[:], scr[:], Act.Copy,
                                             accum_out=G2[:, idx:idx + 1])
            # inv_na20[t, j=(a,th)] = 20/sqrt(G_aa) = sqrt(400/G_aa)
            invr = tmp.tile([128, 2 * A], dt.float32, tag="invr")
            nc.vector.reciprocal(invr[:], G2[:, 0:2 * A])
            invna = gsc.tile([128, 2 * A], dt.float32, tag="invna", name="invna")
            nc.scalar.activation(invna[:], invr[:], Act.Sqrt, scale=SCALE * SCALE)
            # double offdiag G columns (q2 = sum over unordered pairs)
            G2d = gsc.tile([128, 2 * len(_PAIRS)], dt.float32, tag="G2d", name="G2d")
            nc.vector.tensor_scalar(out=G2d[:, 0:2 * A], in0=G2[:, 0:2 * A],
                                    scalar1=1.0, scalar2=None, op0=Alu.mult)
            nc.vector.tensor_scalar(out=G2d[:, 2 * A:], in0=G2[:, 2 * A:],
                                    scalar1=2.0, scalar2=None, op0=Alu.mult)

            # ---- z via v-space: uz = Wa_dv^T nvisD ; z_j = attr_j^T uz ----
            uz = []
            for k in range(KV):
                pt = psum.tile([128, IPC], dt.float32, tag="ps")
                for kd in range(KD):
                    nc.tensor.matmul(pt[:], Wadv[kd][:, k * 128:(k + 1) * 128],
                                     nvisD[kd][:], start=(kd == 0), stop=(kd == KD - 1))
                t = gsc.tile([128, IPC], dt.bfloat16, tag=f"uz{k}", name=f"uz{k}")
                nc.scalar.copy(t[:], pt[:])
                uz.append(t)
            ppool = ph.enter_context(tc.tile_pool(name="pp", bufs=1))
            p_sb, g_sb = [], []
            for j in range(2 * A):
                zp = psum.tile([128, IPC], dt.float32, tag="ps")
                for k in range(KV):
                    nc.tensor.matmul(zp[:], attrT[k][:, j * 128:(j + 1) * 128],
                                     uz[k][:], start=(k == 0), stop=(k == KV - 1))
                e = tmp.tile([128, IPC], dt.float32, tag="eA")
                nc.scalar.activation(e[:], zp[:], Act.Exp, scale=invna[:, j:j + 1])
                p = ppool.tile([128, IPC], dt.float32, tag=f"p{j}", name=f"p{j}")
                nc.vector.tensor_scalar(out=p[:], in0=e[:], scalar1=1.0,
                                        scalar2=mask01[:, j:j + 1], op0=Alu.max,
                                        op1=Alu.mult)
                g = ppool.tile([128, IPC], dt.bfloat16, tag=f"g{j}", name=f"g{j}")
                nc.vector.tensor_tensor(g[:], p[:], zp[:], op=Alu.mult)
                p_sb.append(p)
                g_sb.append(g)
            # num[th] = sum_a g ; q2[th] = sum_pairs Gc*p_a*p_b (identity-mm)
            for th in range(2):
                nump = psB.tile([128, IPC], dt.float32, tag="numP")
                for a in range(A):
                    nc.tensor.matmul(nump[:], identb[:], g_sb[2 * a + th][:],
                                     start=(a == 0), stop=(a == A - 1))
                q2p = psB.tile([128, IPC], dt.float32, tag="q2P")
                with tc.tile_pool(name=f"rp{th}", bufs=4) as rp:
                    for pi, (a, b) in enumerate(_PAIRS):
                        r = rp.tile([128, IPC], dt.bfloat16, tag="rab")
                        nc.vector.scalar_tensor_tensor(
                            out=r[:], in0=p_sb[2 * a + th][:],
                            scalar=G2d[:, 2 * pi + th:2 * pi + th + 1],
                            in1=p_sb[2 * b + th][:], op0=Alu.mult, op1=Alu.mult)
                        nc.tensor.matmul(q2p[:], identb[:], r[:], start=(pi == 0),
                                         stop=(pi == len(_PAIRS) - 1))
                rcq = tmp.tile([128, IPC], dt.float32, tag="rcq")
                nc.vector.reciprocal(rcq[:], q2p[:])
                sq = tmp.tile([128, IPC], dt.float32, tag="sqq")
                nc.scalar.activation(sq[:], rcq[:], Act.Sqrt)  # 1/sqrt(q2)
                nm = tmp.tile([128, IPC], dt.float32, tag="nm")
                nc.vector.tensor_scalar(out=nm[:], in0=nump[:], scalar1=0.1,
                                        scalar2=None, op0=Alu.mult)
                nc.vector.tensor_tensor(i2t_sb[th][:], nm[:], sq[:], op=Alu.mult)

        # =========== phase 2: t2i (streamed blocks of BLK images) ===========
        xpool = top.enter_context(tc.tile_pool(name="xp", bufs=1))
        x_fm = [xpool.tile([128, NP], dt.bfloat16, tag=f"x{k}", name=f"x{k}")
                for k in range(KV)]
        for k in range(KV):
            nc.sync.dma_start(x_fm[k][:], d_x[k * 128:(k + 1) * 128, :])

        with ExitStack() as ph:
            upool = ph.enter_context(tc.tile_pool(name="up", bufs=1))
            psum = ph.enter_context(tc.tile_pool(name="ps2", bufs=6, space="PSUM"))
            psN = ph.enter_context(tc.tile_pool(name="psN", bufs=1, space="PSUM"))
            tmp = ph.enter_context(tc.tile_pool(name="tmp2", bufs=4))
            sp = ph.enter_context(tc.tile_pool(name="sp", bufs=1))

            with tc.tile_pool(name="wpd", bufs=1) as wpd:
                Wpdv = load_w(wpd, d_Wpdv, KD, "wpd")  # [d, v] tiles
                # M = Wp_dv^T Wp_dv  [v, v'] bf16
                M_sb = [upool.tile([128, V], dt.bfloat16, tag=f"M{m}", name=f"M{m}")
                        for m in range(KV)]
                for m in range(KV):
                    for c in range(2):
                        pt = psum.tile([128, 512], dt.float32, tag="ps")
                        for k in range(KD):
                            nc.tensor.matmul(pt[:, 0:384],
                                             Wpdv[k][:, m * 128:(m + 1) * 128],
                                             Wpdv[k][:, c * 384:(c + 1) * 384],
                                             start=(k == 0), stop=(k == KD - 1))
                        nc.vector.tensor_scalar(out=M_sb[m][:, c * 384:(c + 1) * 384],
                                                in0=pt[:, 0:384], scalar1=1.0,
                                                scalar2=None, op0=Alu.mult)
                # u = Wp_dv^T ntextD  [v, t] bf16
                u_sb = [upool.tile([128, BT], dt.bfloat16, tag=f"u{m}", name=f"u{m}")
                        for m in range(KV)]
                for m in range(KV):
                    pt = psum.tile([128, BT], dt.float32, tag="ps")
                    for k in range(KD):
                        nc.tensor.matmul(pt[:], Wpdv[k][:, m * 128:(m + 1) * 128],
                                         ntextD[k][:], start=(k == 0), stop=(k == KD - 1))
                    nc.scalar.copy(u_sb[m][:], pt[:])

            num2p = psN.tile([IPC, BT], dt.float32, tag="num2")
            q2yp = psN.tile([IPC, BT], dt.float32, tag="q2y")
            num2p = num2p[:]
            q2yp = q2yp[:]

            for b in range(NBLK):
                b0 = b * BCOLS
                # t2 block = M x[:, block]  [v', 1568] bf16
                t2 = [sp.tile([128, BCOLS], dt.bfloat16, tag=f"t2{m}", name=f"t2{m}",
                              bufs=1) for m in range(KV)]
                chunks = []
                n0 = 0
                while n0 < BCOLS:
                    chunks.append((n0, min(512, BCOLS - n0)))
                    n0 += 512
                ci = 0
                for m in range(KV):
                    # k outer / chunk inner: each M_sb lhsT loaded once per 4 mm
                    pts = [psum.tile([128, 512], dt.float32, tag="ps", name=f"pc{c}")
                           for c in range(len(chunks))]
                    for k in range(KV):
                        for c, (n0, nw) in enumerate(chunks):
                            nc.tensor.matmul(pts[c][:, 0:nw],
                                             M_sb[k][:, m * 128:(m + 1) * 128],
                                             x_fm[k][:, b0 + n0:b0 + n0 + nw],
                                             start=(k == 0), stop=(k == KV - 1))
                    for c, (n0, nw) in enumerate(chunks):
                        if ci % 2 == 0:
                            nc.vector.tensor_scalar(out=t2[m][:, n0:n0 + nw],
                                                    in0=pts[c][:, 0:nw], scalar1=1.0,
                                                    scalar2=None, op0=Alu.mult)
                        else:
                            nc.scalar.copy(t2[m][:, n0:n0 + nw], pts[c][:, 0:nw])
                        ci += 1
                # Gp per image + diag
                Gp = [sp.tile([128, P], dt.bfloat16, tag=f"gp{t}", name=f"gp{t}",
                              bufs=2) for t in range(2 * BLK)]
                dcol = sp.tile([128, 2 * BLK], dt.float32, tag="dcol", name="dcol",
                               bufs=2)
                nc.gpsimd.memset(dcol[:], 1.0)
                for ib in range(BLK):
                    c0 = b0 + ib * P
                    cb = ib * P
                    for mt in range(2):
                        moff = mt * 128
                        mw = 128 if mt == 0 else P - 128
                        tau = 2 * ib + mt
                        pt = psum.tile([128, 512], dt.float32, tag="ps")
                        for k in range(KV):
                            nc.tensor.matmul(pt[0:mw, 0:P],
                                             x_fm[k][:, c0 + moff:c0 + moff + mw],
                                             t2[k][:, cb:cb + P],
                                             start=(k == 0), stop=(k == KV - 1))
                        if tau % 2 == 0:
                            nc.vector.tensor_scalar(out=Gp[tau][0:mw, :],
                                                    in0=pt[0:mw, 0:P], scalar1=1.0,
                                                    scalar2=None, op0=Alu.mult)
                        else:
                            nc.scalar.copy(Gp[tau][0:mw, :], pt[0:mw, 0:P])
                        dsc = tmp.tile([128, P], dt.bfloat16, tag="dsc", bufs=3)
                        dI = diagE if mt == 0 else diagO
                        meng = nc.vector if tau % 2 == 0 else nc.gpsimd
                        meng.tensor_tensor(dsc[0:mw, :], Gp[tau][0:mw, :],
                                           dI[0:mw, :], op=Alu.mult)
                        if tau % 2 == 0:
                            nc.vector.tensor_reduce(dcol[0:mw, tau:tau + 1],
                                                    dsc[0:mw, :],
                                                    axis=mybir.AxisListType.X,
                                                    op=Alu.add)
                        else:
                            dsc2 = tmp.tile([128, P], dt.bfloat16, tag="dsc2", bufs=3)
                            nc.scalar.activation(dsc2[0:mw, :], dsc[0:mw, :], Act.Copy,
                                                 accum_out=dcol[0:mw, tau:tau + 1])
                # inv20 = 20/sqrt(max(diag,eps)) = sqrt(400*recip(max(diag,eps)))
                dm = tmp.tile([128, 2 * BLK], dt.float32, tag="dm", bufs=2)
                nc.vector.tensor_scalar(out=dm[:], in0=dcol[:], scalar1=1e-12,
                                        scalar2=None, op0=Alu.max)
                dr = tmp.tile([128, 2 * BLK], dt.float32, tag="dr", bufs=2)
                nc.vector.reciprocal(dr[:], dm[:])
                inv20 = tmp.tile([128, 2 * BLK], dt.float32, tag="i20", bufs=2)
                nc.scalar.activation(inv20[:], dr[:], Act.Sqrt, scale=SCALE * SCALE)

                # w -> e/p2/g -> num2 ; r -> g2 -> q2y
                for ib in range(BLK):
                    c0 = b0 + ib * P
                    i = b * BLK + ib
                    es, p2s = [], []
                    for mt in range(2):
                        moff = mt * 128
                        mw = 128 if mt == 0 else P - 128
                        tau = 2 * ib + mt
                        wp = psum.tile([128, 512], dt.float32, tag="ps")
                        for k in range(KV):
                            nc.tensor.matmul(wp[0:mw, 0:BT],
                                             x_fm[k][:, c0 + moff:c0 + moff + mw],
                                             u_sb[k][:], start=(k == 0),
                                             stop=(k == KV - 1))
                        e = tmp.tile([128, BT], dt.float32, tag="e2", bufs=6)
                        nc.scalar.activation(e[0:mw, :], wp[0:mw, 0:BT], Act.Exp,
                                             scale=inv20[0:mw, tau:tau + 1])
                        p2 = sp.tile([128, BT], dt.bfloat16, tag="p2", bufs=6)
                        nc.gpsimd.tensor_scalar(out=p2[0:mw, :], in0=e[0:mw, :],
                                                scalar1=1.0, scalar2=None, op0=Alu.max)
                        g = sp.tile([128, BT], dt.bfloat16, tag="gg", bufs=4)
                        nc.vector.scalar_tensor_tensor(out=g[0:mw, :], in0=e[0:mw, :],
                                                       scalar=1.0, in1=wp[0:mw, 0:BT],
                                                       op0=Alu.max, op1=Alu.mult)
                        nc.tensor.matmul(num2p, IND[0:mw, 31 - i:63 - i], g[0:mw, :],
                                         start=(i == 0 and mt == 0),
                                         stop=(i == IPC - 1 and mt == 1))
                        es.append(e)
                        p2s.append(p2)
                    for mt in range(2):
                        moff = mt * 128
                        mw = 128 if mt == 0 else P - 128
                        rp = psum.tile([128, 512], dt.float32, tag="ps")
                        for k in range(2):
                            kw = 128 if k == 0 else P - 128
                            nc.tensor.matmul(rp[0:mw, 0:BT],
                                             Gp[2 * ib + k][0:kw, moff:moff + mw],
                                             p2s[k][0:kw, :], start=(k == 0),
                                             stop=(k == 1))
                        g2 = sp.tile([128, BT], dt.bfloat16, tag="g2", bufs=4)
                        nc.vector.scalar_tensor_tensor(out=g2[0:mw, :],
                                                       in0=es[mt][0:mw, :], scalar=1.0,
                                                       in1=rp[0:mw, 0:BT],
                                                       op0=Alu.max, op1=Alu.mult)
                        nc.tensor.matmul(q2yp, IND[0:mw, 31 - i:63 - i], g2[0:mw, :],
                                         start=(i == 0 and mt == 0),
                                         stop=(i == IPC - 1 and mt == 1))

            # t2i = num2/sqrt(q2y)  [IPC, BT] -> transpose -> out
            rc = tmp.tile([IPC, BT], dt.float32, tag="rcy")
            nc.vector.reciprocal(rc[:], q2yp)
            sq = tmp.tile([IPC, BT], dt.float32, tag="sqy")
            nc.scalar.activation(sq[:], rc[:], Act.Sqrt)  # 1/sqrt(q2y)
            t2i = tmp.tile([IPC, BT], dt.float32, tag="t2i")
            nc.vector.tensor_tensor(t2i[:], num2p, sq[:], op=Alu.mult)
            for th in range(2):
                tp = psum.tile([128, 512], dt.float32, tag="ps")
                nc.tensor.transpose(tp[0:128, 0:IPC], t2i[:, th * 128:(th + 1) * 128],
                                    identf[0:IPC, 0:IPC])
                nc.vector.tensor_tensor(out_sb[th][:], i2t_sb[th][:], tp[0:128, 0:IPC],
                                        op=Alu.add)
                nc.sync.dma_start(d_out[th * 128:(th + 1) * 128, :], out_sb[th][:])

    nc.compile()
    return nc


def _prep(inputs):
    vf = np.asarray(inputs["visual_feature"], np.float32)
    tf = np.asarray(inputs["textual_feature"], np.float32)
    af = np.asarray(inputs["attribute_feature"], np.float32)
    an = np.asarray(inputs["att_nums"]).astype(np.int64)
    Wp = np.asarray(inputs["Wp"], np.float32)
    Wa = np.asarray(inputs["Wa"], np.float32)
    Wt = np.asarray(inputs["Wt"], np.float32)
    Wv = np.asarray(inputs["Wv"], np.float32)

    textT = np.ascontiguousarray(tf.T).astype(BF16)                       # [768,256]
    attrT = np.ascontiguousarray(af.transpose(1, 0, 2).reshape(A * BT, V).T).astype(BF16)
    WaT = np.ascontiguousarray(Wa.T).astype(BF16)
    WtT = np.ascontiguousarray(Wt.T).astype(BF16)
    WvT = np.ascontiguousarray(Wv.T).astype(BF16)
    Wpdv = np.ascontiguousarray(Wp).astype(BF16)                          # [1024,768]
    Wadv = np.ascontiguousarray(Wa).astype(BF16)
    # mask01 [128, 20]: col j=(a, th) -> 1.0 if a < att_nums[th*128+r]
    m = (np.arange(A)[None, :] < an[:, None]).astype(np.float32)          # [256,10]
    mask01 = np.empty((128, 2 * A), np.float32)
    for a in range(A):
        for th in range(2):
            mask01[:, 2 * a + th] = m[th * 128:(th + 1) * 128, a]

    maps = []
    for c in range(NC_):
        sl = slice(c * IPC, (c + 1) * IPC)
        pat = vf[sl, 1:, :]                                               # [32,196,768]
        patf = np.ascontiguousarray(pat.reshape(NP, V).T).astype(BF16)
        visT = np.ascontiguousarray(vf[sl, 0, :].T).astype(BF16)
        maps.append({
            "patchf": patf, "attr": attrT, "text": textT, "vis": visT,
            "Wpdv": Wpdv, "Wadv": Wadv, "Wa": WaT, "Wt": WtT, "Wv": WvT,
            "mask01": mask01,
        })
    return maps


def _run(inputs, trace=False):
    from concourse.bass_utils import run_bass_kernel_spmd
    if "nc" not in _CACHE:
        _CACHE["nc"] = _build()
    maps = _prep(inputs)
    res = run_bass_kernel_spmd(_CACHE["nc"], maps, list(range(NC_)), trace=trace)
    out = np.concatenate([res.results[c]["out"] for c in range(NC_)], axis=1)
    return out.astype(np.float32), res


def kernel(**inputs):
    out, _ = _run(inputs, trace=False)
    return out
